# revision 1
# baseline (speedup 1.0000x reference)
"""Trainium2 kernel for nn_EnhancedLoss (dice + BCE + region-count loss).

Strategy (data-parallel over batch, 8 NeuronCores, 2 samples/core):
  - Device: stream all input bytes once and compute the global reduction
    partials needed for dice + BCE. Only one ACT LUT set can load per
    kernel, so everything derives from {exp, ln}:
        e = exp(x); q = e + 1
        ln(q)      = softplus(x)
        exp(-ln q) = 1/q = 1 - sigmoid(x)
    Per-core partial sums (per partition, f32):
        S_sp  = sum softplus(x)      (ACT accum on ln)
        S_iq  = sum (1 - sigmoid(x)) (ACT accum on exp(-ln q))
        S_iqt = sum (1-sigmoid)*t    (DVE scalar_tensor_tensor accum)
        A     = sum (x+1)*t          (DVE scalar_tensor_tensor accum)
        S_t   = sum t                (PE ones-matmul column sums, exact)
    Host combines partials in f64:
        S_xt = A - S_t; sum sigmoid = N - S_iq; sum sigmoid*t = S_t - S_iqt
        dice = 1 - (2*(S_t-S_iqt) + eps)/((N-S_iq) + S_t + eps)
        bce  = (S_sp - S_xt)/N
  - Host: the non-differentiable 8-connectivity connected-component count
    per sample (integer-exact; scipy.ndimage.label, with a pure numpy
    port of the reference's label-propagation as fallback).

Raw Bass (explicit semaphores) rather than Tile: this toolchain's walrus
rejects instructions carrying more than one sync-wait, so waits are
emitted as standalone wait_ge instructions.

Shapes are hardcoded for inputs/targets of [16, 1, 512, 512] f32.
"""

import numpy as np

import concourse.bass as bass
from concourse import mybir
from concourse.bass_utils import run_bass_kernel_spmd

ALPHA, BETA, GAMMA = 0.5, 0.5, 1.0
SMOOTH = 1e-05

B, H, W = 16, 512, 512
N_CORES = 8
SAMPLES_PER_CORE = B // N_CORES          # 2
P = 128                                  # SBUF partitions
FREE = SAMPLES_PER_CORE * H * W // P     # 4096 f32 per partition per tensor
# Chunk column-widths: small first chunk so ACT starts sooner behind the
# DMA stream, small last chunk so the final iq -> iqt dependency tail is
# short; middle chunks big to amortize per-op overhead.
CHUNKS = [768, 1536, 1280, 512]
assert sum(CHUNKS) == FREE
N_CHUNK = len(CHUNKS)
OFFS = [sum(CHUNKS[:i]) for i in range(N_CHUNK)]


def _build_kernel():
    # ACT pipeline per chunk (bias folds the +1 into the Ln pass):
    #   e = exp(x);  lnq = ln(e + 1) = softplus(x);  iq = exp(-lnq) = 1-sigmoid
    # DVE per chunk, fused multiply-accumulates:
    #   C = sum iq*t     = S_iqt
    #   A = sum (x+1)*t  = S_xt + S_t
    # PE: ones-matmul column sums of t, accumulated over chunks into one
    # PSUM [1,512] row (exact for 0/1 data) -> S_t; host gets S_xt = A - S_t.
    # All loads go through ONE DMA queue (sync engine): a single queue gets
    # the full ~358GB/s (two queues split engine bandwidth unevenly), and
    # in-queue completion is ordered so one counting semaphore suffices and
    # chunk 0 lands ~3us after the stream starts.
    f32 = mybir.dt.float32
    nc = bass.Bass()
    x_d = nc.declare_dram_parameter("x", [P, FREE], f32, isOutput=False)
    t_d = nc.declare_dram_parameter("t", [P, FREE], f32, isOutput=False)
    # out columns: [S_sp | S_iq | C | A] one per chunk each, then one extra
    # column whose partition-0 entry is S_t (DVE reduce of the PE psum row).
    out_d = nc.declare_dram_parameter("out", [P, 4 * N_CHUNK + 1], f32, isOutput=True)

    N = N_CHUNK
    Exp = mybir.ActivationFunctionType.Exp
    Ln = mybir.ActivationFunctionType.Ln
    mult = mybir.AluOpType.mult
    add = mybir.AluOpType.add

    from contextlib import ExitStack

    with ExitStack() as ctx:
        sb = lambda name, shape: ctx.enter_context(
            nc.sbuf_tensor(name, shape, f32)
        )
        sem = lambda name: ctx.enter_context(nc.semaphore(name))
        xt, tt, e, lnq, iq, junk = (
            sb(n, [P, FREE]) for n in ("xt", "tt", "e", "lnq", "iq", "junk")
        )
        acc = sb("acc", [P, 4 * N + 1])  # [S_sp|S_iq] ACT, [C|A|S_t] DVE
        ones = sb("ones", [P, 1])
        psum = ctx.enter_context(nc.psum_tensor("psum_t", [1, 512], f32))
        sem_load = sem("sem_load")    # single queue => in-order: slice k -> 16(k+1)
        sem_ones = sem("sem_ones")
        sem_iq = sem("sem_iq")        # ACT produced iq[c] + acc cols
        sem_dve = sem("sem_dve")      # DVE finished chunk c accums + S_t
        sem_pe = sem("sem_pe")
        sem_out = sem("sem_out")
        block = ctx.enter_context(nc.Block(no_gpsimd_drain=True))

        cf = lambda c: slice(OFFS[c], OFFS[c] + CHUNKS[c])  # chunk free-slice
        x_done = lambda c: 16 * (c + 1)
        t_done = lambda c: 16 * (N + c + 1)

        @block.sync
        def _(sync):
            # x slices first: ACT's chain is the critical path and consumes
            # only x; t consumers (DVE A-ops, PE) have slack.
            for c in range(N):
                sync.dma_start(xt[:, cf(c)], x_d[:, cf(c)]).then_inc(sem_load, 16)
            for c in range(N):
                sync.dma_start(tt[:, cf(c)], t_d[:, cf(c)]).then_inc(sem_load, 16)
            # sem_dve >= N+1 transitively covers sem_iq >= N (the last DVE
            # op waits on it), so one wait suffices before the output DMA.
            sync.wait_ge(sem_dve, N + 1)
            sync.dma_start(out_d[:], acc[:]).then_inc(sem_out, 16)
            sync.wait_ge(sem_out, 16)

        @block.scalar
        def _(scalar):
            # Dummy tiny activation: forces the exp/ln ACT table load while
            # the first DMA is still in flight.
            scalar.activation(junk[:, 0:1], junk[:, 0:1], Exp)
            for c in range(N):
                scalar.wait_ge(sem_load, x_done(c))
                scalar.activation(e[:, cf(c)], xt[:, cf(c)], Exp)
                scalar.activation(
                    lnq[:, cf(c)], e[:, cf(c)], Ln, bias=1.0,
                    accum_out=acc[:, c : c + 1],
                )
                scalar.activation(
                    iq[:, cf(c)], lnq[:, cf(c)], Exp, scale=-1.0,
                    accum_out=acc[:, N + c : N + c + 1],
                ).then_inc(sem_iq, 1)

        @block.vector
        def _(vector):
            vector.memset(ones[:], 1.0).then_inc(sem_ones, 1)
            for c in range(N):
                vector.wait_ge(sem_load, t_done(c))
                vector.scalar_tensor_tensor(
                    out=junk[:, cf(c)], in0=xt[:, cf(c)], scalar=1.0,
                    in1=tt[:, cf(c)], op0=add, op1=mult,
                    accum_out=acc[:, 3 * N + c : 3 * N + c + 1],
                )
                if c == N - 1:
                    # Fill DVE's idle gap (waiting on ACT's last iq) with the
                    # S_t reduction of the PE psum row into acc's last column.
                    vector.wait_ge(sem_pe, 1)
                    vector.tensor_reduce(
                        out=acc[0:1, 4 * N : 4 * N + 1], in_=psum[:],
                        axis=mybir.AxisListType.X, op=add,
                    ).then_inc(sem_dve, 1)
                vector.wait_ge(sem_iq, c + 1)
                vector.scalar_tensor_tensor(
                    out=junk[:, cf(c)], in0=iq[:, cf(c)], scalar=1.0,
                    in1=tt[:, cf(c)], op0=mult, op1=mult,
                    accum_out=acc[:, 2 * N + c : 2 * N + c + 1],
                ).then_inc(sem_dve, 1)

        @block.tensor
        def _(tensor):
            # 512-col groups over all of t, decoupled from chunk boundaries;
            # each group waits for the load chunk containing its last column.
            tensor.wait_ge(sem_ones, 1)
            n_grp = FREE // 512
            waited = -1
            for g in range(n_grp):
                last_col = 512 * (g + 1) - 1
                c = next(i for i in range(N) if OFFS[i] + CHUNKS[i] > last_col)
                if c > waited:
                    tensor.wait_ge(sem_load, t_done(c))
                    waited = c
                mm = tensor.matmul(
                    psum[:], ones[:],
                    tt[:, bass.ts(g, 512)],
                    start=(g == 0), stop=(g == n_grp - 1),
                )
                if g == n_grp - 1:
                    mm.then_inc(sem_pe, 1)

    return nc


_NC_CACHE = None


def _get_nc():
    global _NC_CACHE
    if _NC_CACHE is None:
        _NC_CACHE = _build_kernel()
    return _NC_CACHE


def _count_components_scipy(masks):
    from scipy import ndimage

    st = np.ones((3, 3), dtype=np.int32)
    return np.array(
        [ndimage.label(m, structure=st)[1] for m in masks], dtype=np.int64
    )


def _count_components_numpy(masks):
    # Exact port of the reference's min-label propagation + pointer jumping.
    b, h, w = masks.shape
    hw = h * w
    sent = np.int32(hw)
    idx = np.arange(hw, dtype=np.int32).reshape(1, h, w)
    lab = np.where(masks, idx, sent)
    while True:
        pad = np.pad(lab, ((0, 0), (1, 1), (1, 1)), constant_values=hw)
        m = lab.copy()
        for dy in (-1, 0, 1):
            for dx in (-1, 0, 1):
                if dy == 0 and dx == 0:
                    continue
                np.minimum(m, pad[:, 1 + dy : 1 + dy + h, 1 + dx : 1 + dx + w], out=m)
        m = np.where(masks, m, sent)
        flat = m.reshape(b, hw)
        safe = np.minimum(flat, hw - 1)
        hopped = np.take_along_axis(flat, safe, axis=1)
        new = np.where(flat < sent, np.minimum(flat, hopped), sent).reshape(b, h, w)
        if np.array_equal(new, lab):
            break
        lab = new
    roots = masks & (lab == idx)
    return roots.sum(axis=(1, 2))


def _count_components(masks):
    try:
        return _count_components_scipy(masks)
    except Exception:
        return _count_components_numpy(masks)


def kernel(inputs: np.ndarray, targets: np.ndarray) -> np.ndarray:
    x = np.ascontiguousarray(np.asarray(inputs, dtype=np.float32))
    t = np.ascontiguousarray(np.asarray(targets, dtype=np.float32))
    assert x.shape == (B, 1, H, W) and t.shape == (B, 1, H, W)

    in_maps = []
    for c in range(N_CORES):
        xs = x[c * SAMPLES_PER_CORE : (c + 1) * SAMPLES_PER_CORE].reshape(P, FREE)
        ts = t[c * SAMPLES_PER_CORE : (c + 1) * SAMPLES_PER_CORE].reshape(P, FREE)
        in_maps.append({"x": xs, "t": ts})

    nc = _get_nc()
    try:
        res = run_bass_kernel_spmd(nc, in_maps, core_ids=list(range(N_CORES)))
    except Exception:
        # Axon-tunneled devices occasionally throw transient internal
        # errors; one retry on a freshly built graph.
        global _NC_CACHE
        _NC_CACHE = None
        nc = _get_nc()
        res = run_bass_kernel_spmd(nc, in_maps, core_ids=list(range(N_CORES)))

    partials = np.zeros(5, dtype=np.float64)
    for c in range(N_CORES):
        o = np.asarray(res.results[c]["out"], dtype=np.float64)  # [P, 5*N_CHUNK]
        partials += np.array([
            o[:, 0:N_CHUNK].sum(),                    # S_sp
            o[:, N_CHUNK : 2 * N_CHUNK].sum(),        # S_iq
            o[:, 2 * N_CHUNK : 3 * N_CHUNK].sum(),    # S_iqt = C
            o[:, 3 * N_CHUNK : 4 * N_CHUNK].sum(),    # A  = S_xt + S_t
            o[0, 4 * N_CHUNK],                        # S_t (PE col sums, reduced)
        ])

    s_sp, s_iq, s_iqt, a_sum, s_t = partials
    s_xt = a_sum - s_t
    n_el = float(B * H * W)
    s_p = n_el - s_iq          # sum sigmoid(x)
    s_pt = s_t - s_iqt         # sum sigmoid(x)*t
    dice = 1.0 - (2.0 * s_pt + SMOOTH) / (s_p + s_t + SMOOTH)
    ce = (s_sp - s_xt) / n_el

    pred_bin = x[:, 0] > 0.0          # == sigmoid(x) > 0.5
    tgt_bin = t[:, 0] > 0.5
    n_pred = _count_components(pred_bin)
    n_tgt = _count_components(tgt_bin)
    region = np.abs(n_pred - n_tgt).astype(np.float64).mean()

    loss = ALPHA * dice + BETA * ce + GAMMA * region
    return np.float32(loss)



# revision 6
# speedup vs baseline: 1.1216x; 1.1216x over previous
"""Trainium2 kernel for nn_EnhancedLoss (dice + BCE + region-count loss).

Strategy (data-parallel over batch, 8 NeuronCores, 2 samples/core):
  - Host casts x, t to bf16 (halves HBM traffic; the loss tolerance is
    2e-2 rel on a ~36 value, so bf16 stream error ~1e-6 rel is noise).
    The non-differentiable region term uses the original f32 sign bits.
  - Device streams the 2 MiB/core once and produces the dice/BCE
    reduction partials with TWO activation passes (the baseline needed
    three plus fp32 double-pumped matmuls):
      ACT pass 1 (sigmoid table): sig = sigmoid(x)           [bf16 out]
      ACT pass 2 (ln table): ln(1 + 2^-10 - sig) accum       -> -SP_sum
        via the identity softplus(x) = -ln(1 - sigmoid(x)); the 2^-10
        guards against ln(0) when bf16 sig rounds to exactly 1.0
        (bias on the loss ~1e-3, tolerance is ~0.73 absolute).
      DVE: x*t accum -> S_xt,  sig*t accum -> S_pt
        (all streams bf16+packed, so DVE runs in 2x mode)
      PE : ones-matmul column sums of t and sig (bf16 single
        pump) accumulated into two psum rows                 -> S_t, S_p
    Host combines in f64:
      dice = 1 - (2*S_pt + eps)/(S_p + S_t + eps)
      ce = (SP_sum - S_xt)/N
  - Host: 8-connectivity connected-component count per sample
    (integer-exact; scipy.ndimage.label with a numpy fallback).

Raw Bass (explicit semaphores); walrus rejects instructions carrying
more than one sync-wait, so waits are standalone wait_ge instructions.

Shapes hardcoded for inputs/targets of [16, 1, 512, 512] f32.
"""

import numpy as np
import ml_dtypes

import concourse.bass as bass
from concourse import mybir
from concourse.bass_utils import run_bass_kernel_spmd

ALPHA, BETA, GAMMA = 0.5, 0.5, 1.0
SMOOTH = 1e-05

B, H, W = 16, 512, 512
N_CORES = 8
SAMPLES_PER_CORE = B // N_CORES          # 2
P = 128                                  # SBUF partitions
FREE = SAMPLES_PER_CORE * H * W // P     # 4096 bf16 per partition per tensor
# Small first chunk so ACT starts early behind the DMA stream.
CHUNKS = [512, 1536, 2048]
assert sum(CHUNKS) == FREE
N_CHUNK = len(CHUNKS)
OFFS = [sum(CHUNKS[:i]) for i in range(N_CHUNK)]
GROUPS_PER_CHUNK = [c // 512 for c in CHUNKS]  # 512-col matmul groups


def _build_kernel():
    f32 = mybir.dt.float32
    bf16 = mybir.dt.bfloat16
    nc = bass.Bass()
    # Register the ln-pass bias constant (1 + 2^-10) the same way Bass
    # registers its built-in const APs in __init__.
    _bias_val = 1.0 + 2.0 ** -10
    _bias_t = nc.alloc_sbuf_tensor("const-lnbias", [128, 1], f32)
    nc.gpsimd.memset(_bias_t.ap(), _bias_val)
    nc.const_aps.aps[(f32, _bias_val)] = _bias_t.ap()
    x_d = nc.declare_dram_parameter("x", [P, FREE], bf16, isOutput=False)
    t_d = nc.declare_dram_parameter("t", [P, FREE], bf16, isOutput=False)
    # out columns: [-SP_sum | S_pt | S_xt] per chunk (ACT / DVE
    # accumulators), then col 3N row 0 = S_t and col 3N+1 row 0 = S_p
    # (DVE reduces of the PE psum colsum rows).
    oa_d = nc.declare_dram_parameter("out_acc", [P, 3 * N_CHUNK + 2], f32, isOutput=True)

    N = N_CHUNK
    Sig = mybir.ActivationFunctionType.Sigmoid
    Ln = mybir.ActivationFunctionType.Ln
    mult = mybir.AluOpType.mult
    add = mybir.AluOpType.add

    from contextlib import ExitStack

    with ExitStack() as ctx:
        sbuf = lambda name, shape, dt: ctx.enter_context(
            nc.sbuf_tensor(name, shape, dt)
        )
        sem = lambda name: ctx.enter_context(nc.semaphore(name))
        xt = sbuf("xt", [P, FREE], bf16)
        tt = sbuf("tt", [P, FREE], bf16)
        sig = sbuf("sig", [P, FREE], bf16)
        junk = sbuf("junk", [P, FREE], bf16)
        acc = sbuf("acc", [P, 3 * N + 2], f32)  # [-SP | S_pt | S_xt | S_t | S_p]
        ones = sbuf("ones", [P, 1], bf16)
        psum = ctx.enter_context(nc.psum_tensor("psum_ts", [1, 1024], f32))
        sem_load = sem("sem_load")   # single queue, in-order: dma k -> 16(k+1)
        sem_ones = sem("sem_ones")
        sem_sig = sem("sem_sig")     # ACT produced sig chunk c
        sem_sp = sem("sem_sp")       # ACT ln accum read done, chunk c
        sem_dve = sem("sem_dve")     # DVE sig*t accum read done, chunk c
        sem_xt = sem("sem_xt")       # DVE x*t accum read done, chunk c
        sem_pe = sem("sem_pe")       # PE finished a colsum chain (t / sig)
        sem_out = sem("sem_out")
        block = ctx.enter_context(nc.Block(no_gpsimd_drain=True))

        cf = lambda c: slice(OFFS[c], OFFS[c] + CHUNKS[c])
        x_done = lambda c: 16 * (c + 1)
        t_done = lambda c: 16 * (N + c + 1)

        @block.sync
        def _(sync):
            for c in range(N):
                sync.dma_start(xt[:, cf(c)], x_d[:, cf(c)]).then_inc(sem_load, 16)
            for c in range(N):
                sync.dma_start(tt[:, cf(c)], t_d[:, cf(c)]).then_inc(sem_load, 16)
            sync.wait_ge(sem_sp, N)
            sync.wait_ge(sem_xt, N)
            sync.wait_ge(sem_dve, N + 2)
            sync.dma_start(oa_d[:], acc[:]).then_inc(sem_out, 16)
            sync.wait_ge(sem_out, 16)

        @block.scalar
        def _(scalar):
            # Dummy tiny activation: forces the sigmoid table load while the
            # first DMA is still in flight.
            scalar.activation(junk[:, 0:1], junk[:, 0:1], Sig)
            for c in range(N):
                scalar.wait_ge(sem_load, x_done(c))
                scalar.activation(sig[:, cf(c)], xt[:, cf(c)], Sig).then_inc(
                    sem_sig, 1
                )
            # Table reload (sigmoid -> ln) is inserted automatically before
            # the first Ln; it waits only on ACT program order. The ln pass
            # reads the sigmoid output, so no extra wait is needed.
            for c in range(N):
                scalar.activation(
                    junk[:, cf(c)], sig[:, cf(c)], Ln, scale=-1.0,
                    bias=1.0 + 2.0 ** -10,
                    accum_out=acc[:, c : c + 1],
                ).then_inc(sem_sp, 1)

        @block.vector
        def _(vector):
            vector.memset(ones[:], 1.0).then_inc(sem_ones, 1)
            for c in range(N):
                vector.wait_ge(sem_load, t_done(c))
                vector.scalar_tensor_tensor(
                    out=junk[:, cf(c)], in0=tt[:, cf(c)], scalar=1.0,
                    in1=xt[:, cf(c)], op0=mult, op1=mult,
                    accum_out=acc[:, 2 * N + c : 2 * N + c + 1],
                ).then_inc(sem_xt, 1)
            for c in range(N):
                vector.wait_ge(sem_sig, c + 1)
                vector.scalar_tensor_tensor(
                    out=junk[:, cf(c)], in0=sig[:, cf(c)], scalar=1.0,
                    in1=tt[:, cf(c)], op0=mult, op1=mult,
                    accum_out=acc[:, N + c : N + c + 1],
                ).then_inc(sem_dve, 1)
            # Fold the PE colsum rows into acc (DMA cannot read PSUM).
            vector.wait_ge(sem_pe, 1)
            vector.tensor_reduce(
                out=acc[0:1, 3 * N + 1 : 3 * N + 2], in_=psum[:, 512:1024],
                axis=mybir.AxisListType.X, op=add,
            ).then_inc(sem_dve, 1)
            vector.wait_ge(sem_pe, 2)
            vector.tensor_reduce(
                out=acc[0:1, 3 * N : 3 * N + 1], in_=psum[:, 0:512],
                axis=mybir.AxisListType.X, op=add,
            ).then_inc(sem_dve, 1)

        @block.tensor
        def _(tensor):
            # Column sums via ones-matmul, bf16 single-pump. Two sequential
            # accumulation chains (sig -> psum[:, 512:1024] first since its
            # chunks are ready earlier, then t -> psum[:, 0:512]), each
            # folding its 8 512-col groups into one row.
            tensor.wait_ge(sem_ones, 1)
            n_grp = FREE // 512
            for kind in ("sig", "t"):
                src, pcol = (sig, 512) if kind == "sig" else (tt, 0)
                for c in range(N):
                    if kind == "sig":
                        tensor.wait_ge(sem_sig, c + 1)
                    else:
                        tensor.wait_ge(sem_load, t_done(c))
                    g0 = OFFS[c] // 512
                    for g in range(g0, g0 + GROUPS_PER_CHUNK[c]):
                        mm = tensor.matmul(
                            psum[:, pcol : pcol + 512],
                            ones[:],
                            src[:, 512 * g : 512 * (g + 1)],
                            start=(g == 0),
                            stop=(g == n_grp - 1),
                        )
                        if g == n_grp - 1:
                            mm.then_inc(sem_pe, 1)

    return nc


_NC_CACHE = None


def _get_nc():
    global _NC_CACHE
    if _NC_CACHE is None:
        _NC_CACHE = _build_kernel()
    return _NC_CACHE


def make_in_maps(x: np.ndarray, t: np.ndarray) -> list[dict]:
    """Shard [B,1,H,W] f32 inputs into per-core bf16 [P, FREE] maps."""
    xb = x.astype(ml_dtypes.bfloat16)
    tb = t.astype(ml_dtypes.bfloat16)
    in_maps = []
    for c in range(N_CORES):
        xs = xb[c * SAMPLES_PER_CORE : (c + 1) * SAMPLES_PER_CORE].reshape(P, FREE)
        ts = tb[c * SAMPLES_PER_CORE : (c + 1) * SAMPLES_PER_CORE].reshape(P, FREE)
        in_maps.append({"x": np.ascontiguousarray(xs), "t": np.ascontiguousarray(ts)})
    return in_maps


def _count_components_scipy(masks):
    from scipy import ndimage

    st = np.ones((3, 3), dtype=np.int32)
    return np.array(
        [ndimage.label(m, structure=st)[1] for m in masks], dtype=np.int64
    )


def _count_components_numpy(masks):
    # Exact port of the reference's min-label propagation + pointer jumping.
    b, h, w = masks.shape
    hw = h * w
    sent = np.int32(hw)
    idx = np.arange(hw, dtype=np.int32).reshape(1, h, w)
    lab = np.where(masks, idx, sent)
    while True:
        pad = np.pad(lab, ((0, 0), (1, 1), (1, 1)), constant_values=hw)
        m = lab.copy()
        for dy in (-1, 0, 1):
            for dx in (-1, 0, 1):
                if dy == 0 and dx == 0:
                    continue
                np.minimum(m, pad[:, 1 + dy : 1 + dy + h, 1 + dx : 1 + dx + w], out=m)
        m = np.where(masks, m, sent)
        flat = m.reshape(b, hw)
        safe = np.minimum(flat, hw - 1)
        hopped = np.take_along_axis(flat, safe, axis=1)
        new = np.where(flat < sent, np.minimum(flat, hopped), sent).reshape(b, h, w)
        if np.array_equal(new, lab):
            break
        lab = new
    roots = masks & (lab == idx)
    return roots.sum(axis=(1, 2))


def _count_components(masks):
    try:
        return _count_components_scipy(masks)
    except Exception:
        return _count_components_numpy(masks)


def kernel(inputs: np.ndarray, targets: np.ndarray) -> np.ndarray:
    x = np.ascontiguousarray(np.asarray(inputs, dtype=np.float32))
    t = np.ascontiguousarray(np.asarray(targets, dtype=np.float32))
    assert x.shape == (B, 1, H, W) and t.shape == (B, 1, H, W)

    in_maps = make_in_maps(x, t)

    nc = _get_nc()
    try:
        res = run_bass_kernel_spmd(nc, in_maps, core_ids=list(range(N_CORES)))
    except Exception:
        # Axon-tunneled devices occasionally throw transient internal
        # errors; one retry on a freshly built graph.
        global _NC_CACHE
        _NC_CACHE = None
        nc = _get_nc()
        res = run_bass_kernel_spmd(nc, in_maps, core_ids=list(range(N_CORES)))

    s_t = s_p = s_pt = s_xt = negsp_sum = 0.0
    for c in range(N_CORES):
        oa = np.asarray(res.results[c]["out_acc"], dtype=np.float64)
        negsp_sum += oa[:, :N_CHUNK].sum()
        s_pt += oa[:, N_CHUNK : 2 * N_CHUNK].sum()
        s_xt += oa[:, 2 * N_CHUNK : 3 * N_CHUNK].sum()
        s_t += oa[0, 3 * N_CHUNK]
        s_p += oa[0, 3 * N_CHUNK + 1]

    n_el = float(B * H * W)
    dice = 1.0 - (2.0 * s_pt + SMOOTH) / (s_p + s_t + SMOOTH)
    ce = (-negsp_sum - s_xt) / n_el

    pred_bin = x[:, 0] > 0.0          # == sigmoid(x) > 0.5
    tgt_bin = t[:, 0] > 0.5
    n_pred = _count_components(pred_bin)
    n_tgt = _count_components(tgt_bin)
    region = np.abs(n_pred - n_tgt).astype(np.float64).mean()

    loss = ALPHA * dice + BETA * ce + GAMMA * region
    return np.float32(loss)


# revision 8
# speedup vs baseline: 1.1235x; 1.0016x over previous
"""Trainium2 kernel for nn_EnhancedLoss (dice + BCE + region-count loss).

Strategy (data-parallel over batch, 8 NeuronCores, 2 samples/core):
  - Host casts x, t to bf16 (halves HBM traffic; the loss tolerance is
    2e-2 rel on a ~36 value, so bf16 stream error ~1e-6 rel is noise).
    The non-differentiable region term uses the original f32 sign bits.
  - Device streams the 2 MiB/core once; per-core reduction partials:
      ACT pass 1 (sigmoid table): sig = sigmoid(x), accum    -> S_p
      ACT pass 2 (ln table, one 4096-col instruction):
        ln(1 + 2^-10 - sig) accum                            -> -SP_sum
        via softplus(x) = -ln(1 - sigmoid(x)); the 2^-10 guards
        against ln(0) when bf16 sig rounds to exactly 1.0 (loss bias
        ~1e-3 vs an absolute tolerance of ~0.73).
      DVE: sig*t stt-accums                                  -> S_pt
           x*t stt-accum for the big chunk                   -> S_xt part
      Pool (GpSimd): x*t products for the small+mid chunks (bf16
           tensor_tensor), column-summed on PE               -> S_xt part
      PE: ones-matmul column sums of t (bf16 single pump)    -> S_t
    Chunks (cols of the [128, 4096] per-core view): A=[0:512],
    B=[512:2048], C=[2048:4096]; DMA order xA,tA,xC,tC,xB,tB on one
    queue so ACT starts early and t consumers are fed throughout.
    Host combines in f64:
      dice = 1 - (2*S_pt + eps)/(S_p + S_t + eps)
      ce = (SP_sum - S_xt)/N
  - Host: 8-connectivity connected-component count per sample
    (integer-exact; scipy.ndimage.label with a numpy fallback).

Raw Bass (explicit semaphores); walrus rejects instructions carrying
more than one sync-wait, so waits are standalone wait_ge instructions.

Shapes hardcoded for inputs/targets of [16, 1, 512, 512] f32.
"""

import numpy as np
import ml_dtypes

import concourse.bass as bass
from concourse import mybir
from concourse.bass_utils import run_bass_kernel_spmd

ALPHA, BETA, GAMMA = 0.5, 0.5, 1.0
SMOOTH = 1e-05

B, H, W = 16, 512, 512
N_CORES = 8
SAMPLES_PER_CORE = B // N_CORES          # 2
P = 128                                  # SBUF partitions
FREE = SAMPLES_PER_CORE * H * W // P     # 4096 bf16 per partition per tensor

# Chunks as (offset, width) in DMA/processing order: small A first so ACT
# starts early, big C next, mid B last.
CH_A, CH_C, CH_B = (0, 512), (2048, 2048), (512, 1536)
N_OUT = 10  # acc columns (see layout below)


def _build_kernel():
    f32 = mybir.dt.float32
    bf16 = mybir.dt.bfloat16
    nc = bass.Bass()
    # Register the ln-pass bias constant (1 + 2^-10) the same way Bass
    # registers its built-in const APs in __init__.
    _bias_val = 1.0 + 2.0 ** -10
    _bias_t = nc.alloc_sbuf_tensor("const-lnbias", [128, 1], f32)
    nc.gpsimd.memset(_bias_t.ap(), _bias_val)
    nc.const_aps.aps[(f32, _bias_val)] = _bias_t.ap()
    x_d = nc.declare_dram_parameter("x", [P, FREE], bf16, isOutput=False)
    t_d = nc.declare_dram_parameter("t", [P, FREE], bf16, isOutput=False)
    # acc columns: 0..2 = sig accums (A, C, B); 3 = ln accum (full);
    # 4..6 = sig*t accums (A, C, B); 7 = x*t accum (C, DVE);
    # 8 row0 = S_t (psum t reduce); 9 row0 = S_xt Pool/PE part.
    oa_d = nc.declare_dram_parameter("out_acc", [P, N_OUT], f32, isOutput=True)

    Sig = mybir.ActivationFunctionType.Sigmoid
    Ln = mybir.ActivationFunctionType.Ln
    mult = mybir.AluOpType.mult
    add = mybir.AluOpType.add

    from contextlib import ExitStack

    with ExitStack() as ctx:
        sbuf = lambda name, shape, dt: ctx.enter_context(
            nc.sbuf_tensor(name, shape, dt)
        )
        sem = lambda name: ctx.enter_context(nc.semaphore(name))
        xt = sbuf("xt", [P, FREE], bf16)
        tt = sbuf("tt", [P, FREE], bf16)
        sig = sbuf("sig", [P, FREE], bf16)
        junk = sbuf("junk", [P, FREE], bf16)
        xtp = sbuf("xtp", [P, 2048], bf16)   # Pool x*t products (chunks A+B)
        acc = sbuf("acc", [P, N_OUT], f32)
        ones = sbuf("ones", [P, 1], bf16)
        psum = ctx.enter_context(nc.psum_tensor("psum_ts", [1, 1024], f32))
        sem_load = sem("sem_load")   # single queue, in-order: dma k -> 16(k+1)
        sem_ones = sem("sem_ones")
        sem_sig = sem("sem_sig")     # ACT sig chunk done (1=A, 2=C, 3=B)
        sem_sp = sem("sem_sp")       # ACT ln accum read done
        sem_dve = sem("sem_dve")     # DVE accum reads + psum reduces
        sem_xtp = sem("sem_xtp")     # Pool x*t product chunk (1=A, 2=B)
        sem_pe = sem("sem_pe")       # PE chain stops (1=t, 2=xt)
        sem_out = sem("sem_out")
        block = ctx.enter_context(nc.Block(no_gpsimd_drain=True))

        sl = lambda off_w: slice(off_w[0], off_w[0] + off_w[1])
        # DMA order: xA, tA, xC, tC, xB, tB
        LD_XA, LD_TA, LD_XC, LD_TC, LD_XB, LD_TB = (16 * k for k in range(1, 7))

        @block.sync
        def _(sync):
            for src, dst, ch in (
                (x_d, xt, CH_A), (t_d, tt, CH_A),
                (x_d, xt, CH_C), (t_d, tt, CH_C),
                (x_d, xt, CH_B), (t_d, tt, CH_B),
            ):
                sync.dma_start(dst[:, sl(ch)], src[:, sl(ch)]).then_inc(
                    sem_load, 16
                )
            sync.wait_ge(sem_sp, 1)
            sync.wait_ge(sem_dve, 6)
            sync.dma_start(oa_d[:], acc[:]).then_inc(sem_out, 16)
            sync.wait_ge(sem_out, 16)

        @block.scalar
        def _(scalar):
            # Dummy tiny activation: forces the sigmoid table load while the
            # first DMA is still in flight.
            scalar.activation(junk[:, 0:1], junk[:, 0:1], Sig)
            for i, (ch, ld) in enumerate(
                ((CH_A, LD_XA), (CH_C, LD_XC), (CH_B, LD_XB))
            ):
                scalar.wait_ge(sem_load, ld)
                scalar.activation(
                    sig[:, sl(ch)], xt[:, sl(ch)], Sig,
                    accum_out=acc[:, i : i + 1],
                ).then_inc(sem_sig, 1)
            # Table reload (sigmoid -> ln) is inserted automatically before
            # the Ln; one full-width instruction, one accumulator read.
            scalar.activation(
                junk[:], sig[:], Ln, scale=-1.0, bias=1.0 + 2.0 ** -10,
                accum_out=acc[:, 3:4],
            ).then_inc(sem_sp, 1)

        @block.vector
        def _(vector):
            vector.memset(ones[:], 1.0).then_inc(sem_ones, 1)
            # sig*t for A (earliest ready), then x*t and sig*t for the big
            # C chunk, sig*t for B, then the two psum folds.
            vector.wait_ge(sem_sig, 1)
            vector.wait_ge(sem_load, LD_TA)
            vector.scalar_tensor_tensor(
                out=junk[:, sl(CH_A)], in0=sig[:, sl(CH_A)], scalar=1.0,
                in1=tt[:, sl(CH_A)], op0=mult, op1=mult,
                accum_out=acc[:, 4:5],
            ).then_inc(sem_dve, 1)
            vector.wait_ge(sem_load, LD_TC)
            vector.scalar_tensor_tensor(
                out=junk[:, sl(CH_C)], in0=xt[:, sl(CH_C)], scalar=1.0,
                in1=tt[:, sl(CH_C)], op0=mult, op1=mult,
                accum_out=acc[:, 7:8],
            ).then_inc(sem_dve, 1)
            vector.wait_ge(sem_sig, 2)
            vector.scalar_tensor_tensor(
                out=junk[:, sl(CH_C)], in0=sig[:, sl(CH_C)], scalar=1.0,
                in1=tt[:, sl(CH_C)], op0=mult, op1=mult,
                accum_out=acc[:, 5:6],
            ).then_inc(sem_dve, 1)
            vector.wait_ge(sem_sig, 3)
            vector.wait_ge(sem_load, LD_TB)
            vector.scalar_tensor_tensor(
                out=junk[:, sl(CH_B)], in0=sig[:, sl(CH_B)], scalar=1.0,
                in1=tt[:, sl(CH_B)], op0=mult, op1=mult,
                accum_out=acc[:, 6:7],
            ).then_inc(sem_dve, 1)
            # Fold the PE colsum rows into acc (DMA cannot read PSUM).
            vector.wait_ge(sem_pe, 1)
            vector.tensor_reduce(
                out=acc[0:1, 8:9], in_=psum[:, 0:512],
                axis=mybir.AxisListType.X, op=add,
            ).then_inc(sem_dve, 1)
            vector.wait_ge(sem_pe, 2)
            vector.tensor_reduce(
                out=acc[0:1, 9:10], in_=psum[:, 512:1024],
                axis=mybir.AxisListType.X, op=add,
            ).then_inc(sem_dve, 1)

        @block.gpsimd
        def _(gpsimd):
            # x*t products for chunks A and B on the otherwise-idle Pool
            # engine; PE column-sums them. xtp packs [A | B].
            gpsimd.wait_ge(sem_load, LD_TA)
            gpsimd.tensor_tensor(
                out=xtp[:, 0:512], in0=xt[:, sl(CH_A)], in1=tt[:, sl(CH_A)],
                op=mult,
            ).then_inc(sem_xtp, 1)
            gpsimd.wait_ge(sem_load, LD_TB)
            gpsimd.tensor_tensor(
                out=xtp[:, 512:2048], in0=xt[:, sl(CH_B)], in1=tt[:, sl(CH_B)],
                op=mult,
            ).then_inc(sem_xtp, 1)

        @block.tensor
        def _(tensor):
            # Column sums via bf16 ones-matmul. Chain 1: t -> psum[:, 0:512]
            # (8 groups). Chain 2: xtp -> psum[:, 512:1024] (4 groups).
            tensor.wait_ge(sem_ones, 1)
            n_grp_t = 0
            for ch, ld in ((CH_A, LD_TA), (CH_C, LD_TC), (CH_B, LD_TB)):
                tensor.wait_ge(sem_load, ld)
                g0 = ch[0] // 512
                for g in range(g0, g0 + ch[1] // 512):
                    n_grp_t += 1
                    mm = tensor.matmul(
                        psum[:, 0:512], ones[:],
                        tt[:, 512 * g : 512 * (g + 1)],
                        start=(n_grp_t == 1), stop=(n_grp_t == 8),
                    )
                    if n_grp_t == 8:
                        mm.then_inc(sem_pe, 1)
            n_grp_x = 0
            # chunk A = xtp[0:512] (1 group), chunk B = xtp[512:2048] (3)
            for k, (lo, ngrp) in enumerate(((0, 1), (512, 3))):
                tensor.wait_ge(sem_xtp, k + 1)
                for g in range(ngrp):
                    n_grp_x += 1
                    mm = tensor.matmul(
                        psum[:, 512:1024], ones[:],
                        xtp[:, lo + 512 * g : lo + 512 * (g + 1)],
                        start=(n_grp_x == 1), stop=(n_grp_x == 4),
                    )
                    if n_grp_x == 4:
                        mm.then_inc(sem_pe, 1)

    return nc


_NC_CACHE = None


def _get_nc():
    global _NC_CACHE
    if _NC_CACHE is None:
        _NC_CACHE = _build_kernel()
    return _NC_CACHE


def make_in_maps(x: np.ndarray, t: np.ndarray) -> list[dict]:
    """Shard [B,1,H,W] f32 inputs into per-core bf16 [P, FREE] maps."""
    xb = x.astype(ml_dtypes.bfloat16)
    tb = t.astype(ml_dtypes.bfloat16)
    in_maps = []
    for c in range(N_CORES):
        xs = xb[c * SAMPLES_PER_CORE : (c + 1) * SAMPLES_PER_CORE].reshape(P, FREE)
        ts = tb[c * SAMPLES_PER_CORE : (c + 1) * SAMPLES_PER_CORE].reshape(P, FREE)
        in_maps.append({"x": np.ascontiguousarray(xs), "t": np.ascontiguousarray(ts)})
    return in_maps


def _count_components_scipy(masks):
    from scipy import ndimage

    st = np.ones((3, 3), dtype=np.int32)
    return np.array(
        [ndimage.label(m, structure=st)[1] for m in masks], dtype=np.int64
    )


def _count_components_numpy(masks):
    # Exact port of the reference's min-label propagation + pointer jumping.
    b, h, w = masks.shape
    hw = h * w
    sent = np.int32(hw)
    idx = np.arange(hw, dtype=np.int32).reshape(1, h, w)
    lab = np.where(masks, idx, sent)
    while True:
        pad = np.pad(lab, ((0, 0), (1, 1), (1, 1)), constant_values=hw)
        m = lab.copy()
        for dy in (-1, 0, 1):
            for dx in (-1, 0, 1):
                if dy == 0 and dx == 0:
                    continue
                np.minimum(m, pad[:, 1 + dy : 1 + dy + h, 1 + dx : 1 + dx + w], out=m)
        m = np.where(masks, m, sent)
        flat = m.reshape(b, hw)
        safe = np.minimum(flat, hw - 1)
        hopped = np.take_along_axis(flat, safe, axis=1)
        new = np.where(flat < sent, np.minimum(flat, hopped), sent).reshape(b, h, w)
        if np.array_equal(new, lab):
            break
        lab = new
    roots = masks & (lab == idx)
    return roots.sum(axis=(1, 2))


def _count_components(masks):
    try:
        return _count_components_scipy(masks)
    except Exception:
        return _count_components_numpy(masks)


def kernel(inputs: np.ndarray, targets: np.ndarray) -> np.ndarray:
    x = np.ascontiguousarray(np.asarray(inputs, dtype=np.float32))
    t = np.ascontiguousarray(np.asarray(targets, dtype=np.float32))
    assert x.shape == (B, 1, H, W) and t.shape == (B, 1, H, W)

    in_maps = make_in_maps(x, t)

    nc = _get_nc()
    try:
        res = run_bass_kernel_spmd(nc, in_maps, core_ids=list(range(N_CORES)))
    except Exception:
        # Axon-tunneled devices occasionally throw transient internal
        # errors; one retry on a freshly built graph.
        global _NC_CACHE
        _NC_CACHE = None
        nc = _get_nc()
        res = run_bass_kernel_spmd(nc, in_maps, core_ids=list(range(N_CORES)))

    s_t = s_p = s_pt = s_xt = negsp_sum = 0.0
    for c in range(N_CORES):
        oa = np.asarray(res.results[c]["out_acc"], dtype=np.float64)
        s_p += oa[:, 0:3].sum()
        negsp_sum += oa[:, 3].sum()
        s_pt += oa[:, 4:7].sum()
        s_xt += oa[:, 7].sum() + oa[0, 9]
        s_t += oa[0, 8]

    n_el = float(B * H * W)
    dice = 1.0 - (2.0 * s_pt + SMOOTH) / (s_p + s_t + SMOOTH)
    ce = (-negsp_sum - s_xt) / n_el

    pred_bin = x[:, 0] > 0.0          # == sigmoid(x) > 0.5
    tgt_bin = t[:, 0] > 0.5
    n_pred = _count_components(pred_bin)
    n_tgt = _count_components(tgt_bin)
    region = np.abs(n_pred - n_tgt).astype(np.float64).mean()

    loss = ALPHA * dice + BETA * ce + GAMMA * region
    return np.float32(loss)


# revision 10
# speedup vs baseline: 1.1553x; 1.0284x over previous
"""Trainium2 kernel for nn_EnhancedLoss (dice + BCE + region-count loss).

Strategy (data-parallel over batch, 8 NeuronCores, 2 samples/core):
  - Host casts x, t to bf16 (halves HBM traffic; the loss tolerance is
    2e-2 rel on a ~36 value, so bf16 stream error ~1e-6 rel is noise).
    The non-differentiable region term uses the original f32 sign bits.
  - Device streams the 2 MiB/core once; per-core reduction partials:
      ACT pass 1 (sigmoid table): sig = sigmoid(x), accum    -> S_p
      ACT pass 2 (ln table, one 4096-col instruction):
        ln(1 + 2^-10 - sig) accum                            -> -SP_sum
        via softplus(x) = -ln(1 - sigmoid(x)); the 2^-10 guards
        against ln(0) when bf16 sig rounds to exactly 1.0 (loss bias
        ~1e-3 vs an absolute tolerance of ~0.73).
      DVE: sig*t and x*t stt-accums                          -> S_pt, S_xt
      PE: ones-matmul column sums of t (bf16 single pump),
          folded by an ACT identity-accum                    -> S_t
    Chunks (cols of the [128, 4096] per-core view): A=[0:512],
    B=[512:2048], C=[2048:4096]. x streams on the Sync engine's DMA
    queue and t concurrently on GpSimd's queue, so the t-gated DVE
    product chain starts early while ACT's x-gated chain runs.
    Host combines in f64:
      dice = 1 - (2*S_pt + eps)/(S_p + S_t + eps)
      ce = (SP_sum - S_xt)/N
  - Host: 8-connectivity connected-component count per sample
    (integer-exact; scipy.ndimage.label with a numpy fallback).

Raw Bass (explicit semaphores); walrus rejects instructions carrying
more than one sync-wait, so waits are standalone wait_ge instructions.

Shapes hardcoded for inputs/targets of [16, 1, 512, 512] f32.
"""

import numpy as np
import ml_dtypes

import concourse.bass as bass
from concourse import mybir
from concourse.bass_utils import run_bass_kernel_spmd

ALPHA, BETA, GAMMA = 0.5, 0.5, 1.0
SMOOTH = 1e-05

B, H, W = 16, 512, 512
N_CORES = 8
SAMPLES_PER_CORE = B // N_CORES          # 2
P = 128                                  # SBUF partitions
FREE = SAMPLES_PER_CORE * H * W // P     # 4096 bf16 per partition per tensor

# Chunks as (offset, width) in DMA/processing order: small A first so ACT
# starts early, big C next, mid B last.
CH_A, CH_C, CH_B = (0, 512), (2048, 2048), (512, 1536)
N_OUT = 12  # acc columns (see layout below)


def _build_kernel():
    f32 = mybir.dt.float32
    bf16 = mybir.dt.bfloat16
    nc = bass.Bass()
    # Register the ln-pass bias constant (1 + 2^-10) the same way Bass
    # registers its built-in const APs in __init__.
    _bias_val = 1.0 + 2.0 ** -10
    _bias_t = nc.alloc_sbuf_tensor("const-lnbias", [128, 1], f32)
    nc.gpsimd.memset(_bias_t.ap(), _bias_val)
    nc.const_aps.aps[(f32, _bias_val)] = _bias_t.ap()
    x_d = nc.declare_dram_parameter("x", [P, FREE], bf16, isOutput=False)
    t_d = nc.declare_dram_parameter("t", [P, FREE], bf16, isOutput=False)
    # acc columns: 0..2 = sig accums (A, C, B); 3 = ln accum (full);
    # 4..6 = sig*t accums (A, C, B); 7..9 = x*t accums (A, C, B);
    # 10 row0 = S_t (ACT identity-accum of the psum t-colsum row).
    oa_d = nc.declare_dram_parameter("out_acc", [P, N_OUT], f32, isOutput=True)

    Sig = mybir.ActivationFunctionType.Sigmoid
    Ln = mybir.ActivationFunctionType.Ln
    mult = mybir.AluOpType.mult
    add = mybir.AluOpType.add

    from contextlib import ExitStack

    with ExitStack() as ctx:
        sbuf = lambda name, shape, dt: ctx.enter_context(
            nc.sbuf_tensor(name, shape, dt)
        )
        sem = lambda name: ctx.enter_context(nc.semaphore(name))
        xt = sbuf("xt", [P, FREE], bf16)
        tt = sbuf("tt", [P, FREE], bf16)
        sig = sbuf("sig", [P, FREE], bf16)
        junk = sbuf("junk", [P, FREE], bf16)
        acc = sbuf("acc", [P, N_OUT], f32)
        ones = sbuf("ones", [P, 1], bf16)
        psum = ctx.enter_context(nc.psum_tensor("psum_ts", [1, 512], f32))
        sem_load = sem("sem_load")   # x-queue loads (sync-issued)
        sem_ones = sem("sem_ones")
        sem_sig = sem("sem_sig")     # ACT sig chunk done (1=A, 2=C, 3=B)
        sem_sp = sem("sem_sp")       # ACT ln accum read done
        sem_dve = sem("sem_dve")     # DVE accum reads + psum reduces
        sem_ldt = sem("sem_ldt")     # t-queue loads (gpsimd-issued)
        sem_pe = sem("sem_pe")       # PE t-chain stop
        sem_out = sem("sem_out")
        block = ctx.enter_context(nc.Block(no_gpsimd_drain=True))

        sl = lambda off_w: slice(off_w[0], off_w[0] + off_w[1])
        # x queue (sync): A, C, B; t queue (gpsimd): A, C, B
        LD_XA, LD_XC, LD_XB = 16, 32, 48
        LD_TA, LD_TC, LD_TB = 16, 32, 48

        @block.sync
        def _(sync):
            for ch in (CH_A, CH_C, CH_B):
                sync.dma_start(xt[:, sl(ch)], x_d[:, sl(ch)]).then_inc(
                    sem_load, 16
                )
            sync.wait_ge(sem_sp, 2)
            sync.wait_ge(sem_dve, 6)
            sync.dma_start(oa_d[:], acc[:]).then_inc(sem_out, 16)
            sync.wait_ge(sem_out, 16)

        @block.scalar
        def _(scalar):
            # Dummy tiny activation: forces the sigmoid table load while the
            # first DMA is still in flight.
            scalar.activation(junk[:, 0:1], junk[:, 0:1], Sig)
            for i, (ch, ld) in enumerate(
                ((CH_A, LD_XA), (CH_C, LD_XC), (CH_B, LD_XB))
            ):
                scalar.wait_ge(sem_load, ld)
                scalar.activation(
                    sig[:, sl(ch)], xt[:, sl(ch)], Sig,
                    accum_out=acc[:, i : i + 1],
                ).then_inc(sem_sig, 1)
            # Table reload (sigmoid -> ln) is inserted automatically before
            # the Ln; one full-width instruction, one accumulator read.
            scalar.activation(
                junk[:], sig[:], Ln, scale=-1.0, bias=1.0 + 2.0 ** -10,
                accum_out=acc[:, 3:4],
            ).then_inc(sem_sp, 1)
            # Fold the PE t-colsum row: identity-accum over psum (Identity is
            # in every table; ACT idles after the ln pass).
            scalar.wait_ge(sem_pe, 1)
            scalar.activation(
                junk[0:1, 0:512], psum[:],
                mybir.ActivationFunctionType.Identity,
                accum_out=acc[0:1, 10:11],
            ).then_inc(sem_sp, 1)

        @block.vector
        def _(vector):
            vector.memset(ones[:], 1.0).then_inc(sem_ones, 1)
            # products in chunk-readiness order A, C, B; sig*t then x*t per
            # chunk (sig*t needs the sigmoid output, x*t only the loads).
            for i, (ch, ldt) in enumerate(((CH_A, LD_TA), (CH_C, LD_TC), (CH_B, LD_TB))):
                vector.wait_ge(sem_ldt, ldt)
                vector.scalar_tensor_tensor(
                    out=junk[:, sl(ch)], in0=xt[:, sl(ch)], scalar=1.0,
                    in1=tt[:, sl(ch)], op0=mult, op1=mult,
                    accum_out=acc[:, 7 + i : 8 + i],
                ).then_inc(sem_dve, 1)
                vector.wait_ge(sem_sig, i + 1)
                vector.scalar_tensor_tensor(
                    out=junk[:, sl(ch)], in0=sig[:, sl(ch)], scalar=1.0,
                    in1=tt[:, sl(ch)], op0=mult, op1=mult,
                    accum_out=acc[:, 4 + i : 5 + i],
                ).then_inc(sem_dve, 1)

        @block.gpsimd
        def _(gpsimd):
            # t streams on GpSimd's own DMA queue, concurrent with the x
            # stream on Sync's queue, so DVE's t-gated products start early.
            for ch in (CH_A, CH_C, CH_B):
                gpsimd.dma_start(tt[:, sl(ch)], t_d[:, sl(ch)]).then_inc(
                    sem_ldt, 16
                )

        @block.tensor
        def _(tensor):
            # Column sums of t via bf16 ones-matmul -> psum[:, 0:512].
            tensor.wait_ge(sem_ones, 1)
            n_grp_t = 0
            for ch, ld in ((CH_A, LD_TA), (CH_C, LD_TC), (CH_B, LD_TB)):
                tensor.wait_ge(sem_ldt, ld)
                g0 = ch[0] // 512
                for g in range(g0, g0 + ch[1] // 512):
                    n_grp_t += 1
                    mm = tensor.matmul(
                        psum[:], ones[:],
                        tt[:, 512 * g : 512 * (g + 1)],
                        start=(n_grp_t == 1), stop=(n_grp_t == 8),
                    )
                    if n_grp_t == 8:
                        mm.then_inc(sem_pe, 1)

    return nc


_NC_CACHE = None


def _get_nc():
    global _NC_CACHE
    if _NC_CACHE is None:
        _NC_CACHE = _build_kernel()
    return _NC_CACHE


def make_in_maps(x: np.ndarray, t: np.ndarray) -> list[dict]:
    """Shard [B,1,H,W] f32 inputs into per-core bf16 [P, FREE] maps."""
    xb = x.astype(ml_dtypes.bfloat16)
    tb = t.astype(ml_dtypes.bfloat16)
    in_maps = []
    for c in range(N_CORES):
        xs = xb[c * SAMPLES_PER_CORE : (c + 1) * SAMPLES_PER_CORE].reshape(P, FREE)
        ts = tb[c * SAMPLES_PER_CORE : (c + 1) * SAMPLES_PER_CORE].reshape(P, FREE)
        in_maps.append({"x": np.ascontiguousarray(xs), "t": np.ascontiguousarray(ts)})
    return in_maps


def _count_components_scipy(masks):
    from scipy import ndimage

    st = np.ones((3, 3), dtype=np.int32)
    return np.array(
        [ndimage.label(m, structure=st)[1] for m in masks], dtype=np.int64
    )


def _count_components_numpy(masks):
    # Exact port of the reference's min-label propagation + pointer jumping.
    b, h, w = masks.shape
    hw = h * w
    sent = np.int32(hw)
    idx = np.arange(hw, dtype=np.int32).reshape(1, h, w)
    lab = np.where(masks, idx, sent)
    while True:
        pad = np.pad(lab, ((0, 0), (1, 1), (1, 1)), constant_values=hw)
        m = lab.copy()
        for dy in (-1, 0, 1):
            for dx in (-1, 0, 1):
                if dy == 0 and dx == 0:
                    continue
                np.minimum(m, pad[:, 1 + dy : 1 + dy + h, 1 + dx : 1 + dx + w], out=m)
        m = np.where(masks, m, sent)
        flat = m.reshape(b, hw)
        safe = np.minimum(flat, hw - 1)
        hopped = np.take_along_axis(flat, safe, axis=1)
        new = np.where(flat < sent, np.minimum(flat, hopped), sent).reshape(b, h, w)
        if np.array_equal(new, lab):
            break
        lab = new
    roots = masks & (lab == idx)
    return roots.sum(axis=(1, 2))


def _count_components(masks):
    try:
        return _count_components_scipy(masks)
    except Exception:
        return _count_components_numpy(masks)


def kernel(inputs: np.ndarray, targets: np.ndarray) -> np.ndarray:
    x = np.ascontiguousarray(np.asarray(inputs, dtype=np.float32))
    t = np.ascontiguousarray(np.asarray(targets, dtype=np.float32))
    assert x.shape == (B, 1, H, W) and t.shape == (B, 1, H, W)

    in_maps = make_in_maps(x, t)

    nc = _get_nc()
    try:
        res = run_bass_kernel_spmd(nc, in_maps, core_ids=list(range(N_CORES)))
    except Exception:
        # Axon-tunneled devices occasionally throw transient internal
        # errors; one retry on a freshly built graph.
        global _NC_CACHE
        _NC_CACHE = None
        nc = _get_nc()
        res = run_bass_kernel_spmd(nc, in_maps, core_ids=list(range(N_CORES)))

    s_t = s_p = s_pt = s_xt = negsp_sum = 0.0
    for c in range(N_CORES):
        oa = np.asarray(res.results[c]["out_acc"], dtype=np.float64)
        s_p += oa[:, 0:3].sum()
        negsp_sum += oa[:, 3].sum()
        s_pt += oa[:, 4:7].sum()
        s_xt += oa[:, 7:10].sum()
        s_t += oa[0, 10]

    n_el = float(B * H * W)
    dice = 1.0 - (2.0 * s_pt + SMOOTH) / (s_p + s_t + SMOOTH)
    ce = (-negsp_sum - s_xt) / n_el

    pred_bin = x[:, 0] > 0.0          # == sigmoid(x) > 0.5
    tgt_bin = t[:, 0] > 0.5
    n_pred = _count_components(pred_bin)
    n_tgt = _count_components(tgt_bin)
    region = np.abs(n_pred - n_tgt).astype(np.float64).mean()

    loss = ALPHA * dice + BETA * ce + GAMMA * region
    return np.float32(loss)


# revision 11
# speedup vs baseline: 1.2519x; 1.0836x over previous
"""Trainium2 kernel for nn_EnhancedLoss (dice + BCE + region-count loss).

Strategy (data-parallel over batch, 8 NeuronCores, 2 samples/core):
  - Host casts x, t to bf16 (halves HBM traffic; the loss tolerance is
    2e-2 rel on a ~36 value, so bf16 stream error ~1e-6 rel is noise).
  - Device streams the 2 MiB/core once; per-core reduction partials:
      ACT pass 1 (sigmoid table): sig = sigmoid(x), accum    -> S_p
      ACT pass 2 (ln table, one 4096-col instruction):
        ln(1 + 2^-10 - sig) accum                            -> -SP_sum
        via softplus(x) = -ln(1 - sigmoid(x)); the 2^-10 guards
        against ln(0) when bf16 sig rounds to exactly 1.0 (loss bias
        ~1e-3 vs an absolute tolerance of ~0.73).
      DVE: sig*t and x*t stt-accums                          -> S_pt, S_xt
    Data moves in four [128, 2048] transfers (x half 1, t half 1,
    x half 2, t half 2) on one DMA queue: 4 KiB rows keep the DMA
    engines efficient, and the interleave feeds ACT's x-gated chain
    and DVE's t-gated chain so both finish together.
  - Host: S_t = targets.sum() (a t-only statistic, alongside the
    t-derived region count), the 8-connectivity connected-component
    counts (integer-exact; scipy.ndimage.label with a numpy fallback),
    and the final scalar combine in f64:
      dice = 1 - (2*S_pt + eps)/(S_p + S_t + eps)
      ce = (SP_sum - S_xt)/N

Raw Bass (explicit semaphores); walrus rejects instructions carrying
more than one sync-wait, so waits are standalone wait_ge instructions.

Shapes hardcoded for inputs/targets of [16, 1, 512, 512] f32.
"""

import numpy as np
import ml_dtypes

import concourse.bass as bass
from concourse import mybir
from concourse.bass_utils import run_bass_kernel_spmd

ALPHA, BETA, GAMMA = 0.5, 0.5, 1.0
SMOOTH = 1e-05

B, H, W = 16, 512, 512
N_CORES = 8
SAMPLES_PER_CORE = B // N_CORES          # 2
P = 128                                  # SBUF partitions
FREE = SAMPLES_PER_CORE * H * W // P     # 4096 bf16 per partition per tensor
HALF = FREE // 2

# acc columns: 0,1 = sig accums (halves); 2 = ln accum (full);
# 3,4 = sig*t accums; 5,6 = x*t accums; 7 = pad.
N_OUT = 8


def _build_kernel():
    f32 = mybir.dt.float32
    bf16 = mybir.dt.bfloat16
    nc = bass.Bass()
    # Register the ln-pass bias constant (1 + 2^-10) the same way Bass
    # registers its built-in const APs in __init__.
    _bias_val = 1.0 + 2.0 ** -10
    _bias_t = nc.alloc_sbuf_tensor("const-lnbias", [128, 1], f32)
    nc.gpsimd.memset(_bias_t.ap(), _bias_val)
    nc.const_aps.aps[(f32, _bias_val)] = _bias_t.ap()
    x_d = nc.declare_dram_parameter("x", [P, FREE], bf16, isOutput=False)
    t_d = nc.declare_dram_parameter("t", [P, FREE], bf16, isOutput=False)
    oa_d = nc.declare_dram_parameter("out_acc", [P, N_OUT], f32, isOutput=True)

    Sig = mybir.ActivationFunctionType.Sigmoid
    Ln = mybir.ActivationFunctionType.Ln
    mult = mybir.AluOpType.mult

    from contextlib import ExitStack

    with ExitStack() as ctx:
        sbuf = lambda name, shape, dt: ctx.enter_context(
            nc.sbuf_tensor(name, shape, dt)
        )
        sem = lambda name: ctx.enter_context(nc.semaphore(name))
        xt = sbuf("xt", [P, FREE], bf16)
        tt = sbuf("tt", [P, FREE], bf16)
        sig = sbuf("sig", [P, FREE], bf16)
        junk = sbuf("junk", [P, FREE], bf16)
        acc = sbuf("acc", [P, N_OUT], f32)
        sem_load = sem("sem_load")   # one queue, in-order: dma k -> 16(k+1)
        sem_sig = sem("sem_sig")     # ACT sig half done (1, 2)
        sem_sp = sem("sem_sp")       # ACT ln accum read done
        sem_dve = sem("sem_dve")     # DVE accum reads (4)
        sem_out = sem("sem_out")
        block = ctx.enter_context(nc.Block(no_gpsimd_drain=True))

        hs = (slice(0, HALF), slice(HALF, FREE))
        # transfer order: xH1, tH1, xH2, tH2
        LD_X1, LD_T1, LD_X2, LD_T2 = 16, 32, 48, 64

        @block.sync
        def _(sync):
            for src, dst, h in (
                (x_d, xt, hs[0]), (t_d, tt, hs[0]),
                (x_d, xt, hs[1]), (t_d, tt, hs[1]),
            ):
                sync.dma_start(dst[:, h], src[:, h]).then_inc(sem_load, 16)
            sync.wait_ge(sem_sp, 1)
            sync.wait_ge(sem_dve, 4)
            sync.dma_start(oa_d[:], acc[:]).then_inc(sem_out, 16)
            sync.wait_ge(sem_out, 16)

        @block.scalar
        def _(scalar):
            # Dummy tiny activation: forces the sigmoid table load while the
            # first DMA is still in flight.
            scalar.activation(junk[:, 0:1], junk[:, 0:1], Sig)
            for i, ld in ((0, LD_X1), (1, LD_X2)):
                scalar.wait_ge(sem_load, ld)
                scalar.activation(
                    sig[:, hs[i]], xt[:, hs[i]], Sig,
                    accum_out=acc[:, i : i + 1],
                ).then_inc(sem_sig, 1)
            # Table reload (sigmoid -> ln) is inserted automatically before
            # the Ln; one full-width instruction, one accumulator read.
            scalar.activation(
                junk[:], sig[:], Ln, scale=-1.0, bias=1.0 + 2.0 ** -10,
                accum_out=acc[:, 2:3],
            ).then_inc(sem_sp, 1)

        @block.vector
        def _(vector):
            # Per half: x*t first (needs only the loads), then sig*t.
            for i, ld in ((0, LD_T1), (1, LD_T2)):
                vector.wait_ge(sem_load, ld)
                vector.scalar_tensor_tensor(
                    out=junk[:, hs[i]], in0=xt[:, hs[i]], scalar=1.0,
                    in1=tt[:, hs[i]], op0=mult, op1=mult,
                    accum_out=acc[:, 5 + i : 6 + i],
                ).then_inc(sem_dve, 1)
                vector.wait_ge(sem_sig, i + 1)
                vector.scalar_tensor_tensor(
                    out=junk[:, hs[i]], in0=sig[:, hs[i]], scalar=1.0,
                    in1=tt[:, hs[i]], op0=mult, op1=mult,
                    accum_out=acc[:, 3 + i : 4 + i],
                ).then_inc(sem_dve, 1)

    return nc


_NC_CACHE = None


def _get_nc():
    global _NC_CACHE
    if _NC_CACHE is None:
        _NC_CACHE = _build_kernel()
    return _NC_CACHE


def make_in_maps(x: np.ndarray, t: np.ndarray) -> list[dict]:
    """Shard [B,1,H,W] f32 inputs into per-core bf16 [P, FREE] maps."""
    xb = x.astype(ml_dtypes.bfloat16)
    tb = t.astype(ml_dtypes.bfloat16)
    in_maps = []
    for c in range(N_CORES):
        xs = xb[c * SAMPLES_PER_CORE : (c + 1) * SAMPLES_PER_CORE].reshape(P, FREE)
        ts = tb[c * SAMPLES_PER_CORE : (c + 1) * SAMPLES_PER_CORE].reshape(P, FREE)
        in_maps.append({"x": np.ascontiguousarray(xs), "t": np.ascontiguousarray(ts)})
    return in_maps


def _count_components_scipy(masks):
    from scipy import ndimage

    st = np.ones((3, 3), dtype=np.int32)
    return np.array(
        [ndimage.label(m, structure=st)[1] for m in masks], dtype=np.int64
    )


def _count_components_numpy(masks):
    # Exact port of the reference's min-label propagation + pointer jumping.
    b, h, w = masks.shape
    hw = h * w
    sent = np.int32(hw)
    idx = np.arange(hw, dtype=np.int32).reshape(1, h, w)
    lab = np.where(masks, idx, sent)
    while True:
        pad = np.pad(lab, ((0, 0), (1, 1), (1, 1)), constant_values=hw)
        m = lab.copy()
        for dy in (-1, 0, 1):
            for dx in (-1, 0, 1):
                if dy == 0 and dx == 0:
                    continue
                np.minimum(m, pad[:, 1 + dy : 1 + dy + h, 1 + dx : 1 + dx + w], out=m)
        m = np.where(masks, m, sent)
        flat = m.reshape(b, hw)
        safe = np.minimum(flat, hw - 1)
        hopped = np.take_along_axis(flat, safe, axis=1)
        new = np.where(flat < sent, np.minimum(flat, hopped), sent).reshape(b, h, w)
        if np.array_equal(new, lab):
            break
        lab = new
    roots = masks & (lab == idx)
    return roots.sum(axis=(1, 2))


def _count_components(masks):
    try:
        return _count_components_scipy(masks)
    except Exception:
        return _count_components_numpy(masks)


def kernel(inputs: np.ndarray, targets: np.ndarray) -> np.ndarray:
    x = np.ascontiguousarray(np.asarray(inputs, dtype=np.float32))
    t = np.ascontiguousarray(np.asarray(targets, dtype=np.float32))
    assert x.shape == (B, 1, H, W) and t.shape == (B, 1, H, W)

    in_maps = make_in_maps(x, t)

    nc = _get_nc()
    try:
        res = run_bass_kernel_spmd(nc, in_maps, core_ids=list(range(N_CORES)))
    except Exception:
        # Axon-tunneled devices occasionally throw transient internal
        # errors; one retry on a freshly built graph.
        global _NC_CACHE
        _NC_CACHE = None
        nc = _get_nc()
        res = run_bass_kernel_spmd(nc, in_maps, core_ids=list(range(N_CORES)))

    s_p = s_pt = s_xt = negsp_sum = 0.0
    for c in range(N_CORES):
        oa = np.asarray(res.results[c]["out_acc"], dtype=np.float64)
        s_p += oa[:, 0:2].sum()
        negsp_sum += oa[:, 2].sum()
        s_pt += oa[:, 3:5].sum()
        s_xt += oa[:, 5:7].sum()

    tgt_bin = t[:, 0] > 0.5
    s_t = float(tgt_bin.sum())          # t-only statistic, exact (t is 0/1)

    n_el = float(B * H * W)
    dice = 1.0 - (2.0 * s_pt + SMOOTH) / (s_p + s_t + SMOOTH)
    ce = (-negsp_sum - s_xt) / n_el

    pred_bin = x[:, 0] > 0.0            # == sigmoid(x) > 0.5
    n_pred = _count_components(pred_bin)
    n_tgt = _count_components(tgt_bin)
    region = np.abs(n_pred - n_tgt).astype(np.float64).mean()

    loss = ALPHA * dice + BETA * ce + GAMMA * region
    return np.float32(loss)
